# revision 12
# baseline (speedup 1.0000x reference)
"""Trainium2 Bass kernel for nn_NeuralCRFTagger (loss_fn).

Strategy
--------
The per-token network em = tanh(emb[v] @ W_feat + b_feat) @ W_emit + b_emit
depends only on the vocab id v, so the whole encoder collapses into a
vocab-indexed emission table.

Phase A (8 cores, vocab row-sharded): build table[v, 0:48] = emissions,
table[v, 48] = sum over tags (the quantity feeding the log-normalizer
recurrence). bf16, rows padded to 128 cols (256B, dma_gather granularity).
Host concatenates the 8 shards (pure marshaling, no math).

Phase B (8 cores, batch-sharded 32 examples/core): dma_gather the emission
row of every token (two gathers + predicated merge since int16 indices cap
at 32767); dma_gather transition rows T[tgt[s+1], :] (with a second table
bank T + start_trans used at s=0, folding the start score in); build
onehot(target) masks with iota/is_equal; reduce gold-path sums with the
ACT accumulator; run the log-normalizer recurrence a' = 48*a + c with a
single fp32 tensor_tensor_scan (the reference's fp32 recurrence overflows
to +/-inf by step ~22 and saturates, so the first 128 steps already yield
the exact +/-inf the reference produces).
"""
import numpy as np

import concourse.bass as bass
import concourse.bacc as bacc
import concourse.tile as tile
from concourse import mybir, library_config
from concourse.bass_utils import run_bass_kernel_spmd
from concourse.masks import make_identity
from contextlib import ExitStack

F32 = mybir.dt.float32
BF16 = mybir.dt.bfloat16
I32 = mybir.dt.int32
I16 = mybir.dt.int16
BF16_NP = np.dtype("bfloat16") if hasattr(np, "bfloat16") else None
if BF16_NP is None:
    import ml_dtypes
    BF16_NP = np.dtype(ml_dtypes.bfloat16)

# Problem shapes (hardcoded per contest contract)
VOCAB, EMB, HID, T, B, S = 50000, 256, 512, 48, 256, 512
NCORES = 8
BPC = B // NCORES            # 32 examples per core
VSH = 6656                   # vocab shard rows per core (13 * 512)
VPAD = VSH * NCORES          # 53248 padded vocab
TBL = 64                     # computed table cols (48 tags + tag-sum + pad)
ROWW = 128                   # stored table row width (bf16 -> 256B rows)
NG = VSH // 512              # 13 groups of 512 vocab rows per core
NTOK = BPC * S               # 16384 tokens per core
HI0 = 32768                  # int16 index split point

_CACHE = {}


def _build_phase_a():
    nc = bacc.Bacc("TRN2", target_bir_lowering=False, debug=False)
    embshT = nc.dram_tensor("embshT", [EMB, VSH], BF16, kind="ExternalInput").ap()
    wfeat = nc.dram_tensor("wfeat", [128, 2, HID], BF16, kind="ExternalInput").ap()
    wemit = nc.dram_tensor("wemit", [128, 4, TBL], BF16, kind="ExternalInput").ap()
    bfeat = nc.dram_tensor("bfeat", [128, 4], F32, kind="ExternalInput").ap()
    bemit = nc.dram_tensor("bemit", [TBL, 1], F32, kind="ExternalInput").ap()
    tablesh = nc.dram_tensor("tablesh", [VSH, ROWW], BF16, kind="ExternalOutput").ap()

    with tile.TileContext(nc) as tc, ExitStack() as ctx:
        cpool = ctx.enter_context(tc.tile_pool(name="const", bufs=1))
        hpool = ctx.enter_context(tc.tile_pool(name="h", bufs=2))
        opool = ctx.enter_context(tc.tile_pool(name="o", bufs=3))
        ph = ctx.enter_context(tc.tile_pool(name="ph", bufs=2, space="PSUM"))
        pe = ctx.enter_context(tc.tile_pool(name="pe", bufs=2, space="PSUM"))
        pt = ctx.enter_context(tc.tile_pool(name="pt", bufs=2, space="PSUM"))

        wf_sb = cpool.tile([128, 2, HID], BF16)
        nc.sync.dma_start(wf_sb[:], wfeat[:])
        we_sb = cpool.tile([128, 4, TBL], BF16)
        nc.sync.dma_start(we_sb[:], wemit[:])
        bfe_sb = cpool.tile([128, 4], F32)
        nc.sync.dma_start(bfe_sb[:], bfeat[:])
        bem_sb = cpool.tile([TBL, 1], F32)
        nc.sync.dma_start(bem_sb[:], bemit[:])
        ident = cpool.tile([128, 128], BF16)
        make_identity(nc, ident[:])

        xt_sb = cpool.tile([128, 2, VSH], BF16)
        for c in range(2):
            nc.sync.dma_start(xt_sb[:, c, :], embshT[c * 128:(c + 1) * 128, :])

        for g in range(NG):
            h_sb = hpool.tile([128, 4, 512], BF16, tag="h")
            for ch in range(4):
                psh = ph.tile([128, 512], F32, tag="ph")
                for ce in range(2):
                    nc.tensor.matmul(
                        psh[:],
                        lhsT=wf_sb[:, ce, ch * 128:(ch + 1) * 128],
                        rhs=xt_sb[:, ce, g * 512:(g + 1) * 512],
                        start=(ce == 0), stop=(ce == 1),
                    )
                nc.scalar.activation(
                    out=h_sb[:, ch, :], in_=psh[:],
                    func=mybir.ActivationFunctionType.Tanh,
                    bias=bfe_sb[:, ch:ch + 1], scale=1.0,
                )
            pse = pe.tile([TBL, 512], F32, tag="pe")
            for ch in range(4):
                nc.tensor.matmul(
                    pse[:], lhsT=we_sb[:, ch, :], rhs=h_sb[:, ch, :],
                    start=(ch == 0), stop=(ch == 3),
                )
            emT_sb = opool.tile([TBL, 512], BF16, tag="emT")
            nc.vector.tensor_scalar(
                out=emT_sb[:], in0=pse[:], scalar1=bem_sb[:], scalar2=None,
                op0=mybir.AluOpType.add,
            )
            pst = pt.tile([128, 4, TBL], BF16, tag="pt")
            for q in range(4):
                nc.tensor.transpose(
                    out=pst[:, q, :], in_=emT_sb[:, q * 128:(q + 1) * 128],
                    identity=ident[0:TBL, 0:TBL],
                )
            tbl_sb = opool.tile([128, 4, TBL], BF16, tag="tbl")
            nc.vector.tensor_copy(tbl_sb[:], pst[:])
            nc.sync.dma_start(
                tablesh[g * 512:(g + 1) * 512, 0:TBL].rearrange(
                    "(q p) e -> p q e", q=4, p=128),
                tbl_sb[:],
            )

    nc.compile()
    return nc


def _build_phase_b():
    nc = bacc.Bacc("TRN2", target_bir_lowering=False, debug=False)
    table = nc.dram_tensor("table", [VPAD, ROWW], BF16, kind="ExternalInput").ap()
    seqT = nc.dram_tensor("seqT", [S, BPC], I32, kind="ExternalInput").ap()
    tgtT = nc.dram_tensor("tgtT", [S, BPC], I32, kind="ExternalInput").ap()
    sqlo = nc.dram_tensor("sqlo", [128, NTOK // 16], I16, kind="ExternalInput").ap()
    sqhi = nc.dram_tensor("sqhi", [128, NTOK // 16], I16, kind="ExternalInput").ap()
    tgn = nc.dram_tensor("tgn", [128, NTOK // 16], I16, kind="ExternalInput").ap()
    transp = nc.dram_tensor("transp", [2 * (T + 1), TBL], F32,
                            kind="ExternalInput").ap()
    misc = nc.dram_tensor("misc", [128, 2], F32, kind="ExternalInput").ap()
    logzraw = nc.dram_tensor("logzraw", [128], F32, kind="ExternalOutput").ap()
    gold = nc.dram_tensor("gold", [BPC], F32, kind="ExternalOutput").ap()

    GB = 16                  # examples per gather batch
    NQ = GB * 4              # row-chunks per batch
    NI = GB * S              # idxs per batch

    with tile.TileContext(nc) as tc, ExitStack() as ctx:
        cpool = ctx.enter_context(tc.tile_pool(name="const", bufs=1))
        gpool = ctx.enter_context(tc.tile_pool(name="g", bufs=2))
        wpool = ctx.enter_context(tc.tile_pool(name="w", bufs=3))
        pp = ctx.enter_context(tc.tile_pool(name="pp", bufs=1, space="PSUM"))

        nc.gpsimd.load_library(library_config.mlp)

        sq_sb = cpool.tile([128, 4, BPC], I32)
        nc.sync.dma_start(sq_sb[:], seqT.rearrange("(j p) b -> p j b", p=128))
        tg_sb = cpool.tile([128, 4, BPC], I32)
        nc.sync.dma_start(tg_sb[:], tgtT.rearrange("(j p) b -> p j b", p=128))
        lo16 = cpool.tile([128, NTOK // 16], I16)
        nc.sync.dma_start(lo16[:], sqlo[:])
        hi16 = cpool.tile([128, NTOK // 16], I16)
        nc.sync.dma_start(hi16[:], sqhi[:])
        tn16 = cpool.tile([128, NTOK // 16], I16)
        nc.sync.dma_start(tn16[:], tgn[:])
        msc = cpool.tile([128, 2], F32)
        nc.sync.dma_start(msc[:], misc[:])

        io_i = cpool.tile([128, 4, T], I32)
        nc.gpsimd.iota(io_i[:], pattern=[[0, 4], [1, T]], base=0,
                       channel_multiplier=0)
        io_bf = cpool.tile([128, 4, T], BF16)
        nc.vector.tensor_copy(io_bf[:], io_i[:])
        tgt_bf = cpool.tile([128, 4, BPC], BF16)
        nc.vector.tensor_copy(tgt_bf[:], tg_sb[:])
        seq_f = cpool.tile([128, 4, BPC], F32)
        nc.vector.tensor_copy(seq_f[:], sq_sb[:])
        pad = cpool.tile([128, 4, BPC], F32)
        nc.vector.tensor_scalar(out=pad[:], in0=seq_f[:], scalar1=0.0,
                                scalar2=None, op0=mybir.AluOpType.is_equal)
        keep_bf = cpool.tile([128, 4, BPC], BF16)
        nc.vector.tensor_scalar(out=keep_bf[:], in0=pad[:], scalar1=-1.0,
                                scalar2=1.0, op0=mybir.AluOpType.mult,
                                op1=mybir.AluOpType.add)
        keep_f = cpool.tile([128, 4, BPC], F32)
        nc.vector.tensor_copy(keep_f[:], keep_bf[:])
        him = cpool.tile([128, 4, BPC], I16)
        nc.vector.tensor_scalar(out=him[:], in0=seq_f[:], scalar1=float(HI0),
                                scalar2=None, op0=mybir.AluOpType.is_ge)

        ones = cpool.tile([128, 1], F32)
        nc.vector.memset(ones[:], 1.0)
        d48 = cpool.tile([128, 128], F32)
        nc.vector.memset(d48[:], float(T))
        identf = cpool.tile([128, 128], F32)
        make_identity(nc, identf[:])

        estack = cpool.tile([128, BPC], F32)
        tstack = cpool.tile([128, BPC], F32)
        cstack = cpool.tile([128, 128], F32)

        for bb in range(BPC // GB):
            b0 = bb * GB
            i0 = b0 * S // 16    # idx-column offset for this batch
            glo = gpool.tile([128, NQ, ROWW], BF16, tag="glo")
            nc.gpsimd.dma_gather(
                out_ap=glo[:], in_ap=table[:], idxs_ap=lo16[:, i0:i0 + NI // 16],
                num_idxs=NI, num_idxs_reg=NI, elem_size=ROWW,
                single_packet=False,
            )
            ghi = gpool.tile([128, NQ, ROWW], BF16, tag="ghi")
            nc.gpsimd.dma_gather(
                out_ap=ghi[:], in_ap=table[HI0:, :], idxs_ap=hi16[:, i0:i0 + NI // 16],
                num_idxs=NI, num_idxs_reg=NI, elem_size=ROWW,
                single_packet=False,
            )
            r8 = gpool.tile([128, NQ, TBL], F32, tag="r8")
            nc.gpsimd.dma_gather(
                out_ap=r8[:], in_ap=transp[:], idxs_ap=tn16[:, i0:i0 + NI // 16],
                num_idxs=NI, num_idxs_reg=NI, elem_size=TBL,
                single_packet=False,
            )
            # merge hi rows over lo rows where seq >= HI0 (only cols 0:64 used)
            him_q = wpool.tile([128, GB, 4], I16, tag="him_q")
            nc.vector.tensor_copy(
                him_q[:], him[:, :, b0:b0 + GB].rearrange("p j b -> p b j"))
            nc.vector.copy_predicated(
                out=glo[:, :, 0:TBL],
                mask=him_q[:].rearrange("p b j -> p (b j)")
                    [:, :, None].to_broadcast([128, NQ, TBL]),
                data=ghi[:, :, 0:TBL],
            )
            em4 = glo[:].rearrange("p (b j) e -> p b j e", j=4)

            # onehot(target) and keep-masked onehot, all examples in batch
            oh16 = wpool.tile([128, GB, 4, T], BF16, tag="oh16")
            nc.vector.tensor_tensor(
                out=oh16[:],
                in0=io_bf[:, None, :, :].to_broadcast([128, GB, 4, T]),
                in1=tgt_bf[:, :, b0:b0 + GB].rearrange("p j b -> p b j")
                    [:, :, :, None].to_broadcast([128, GB, 4, T]),
                op=mybir.AluOpType.is_equal,
            )
            ohm16 = wpool.tile([128, GB, 4, T], BF16, tag="ohm16")
            nc.vector.tensor_tensor(
                out=ohm16[:], in0=oh16[:],
                in1=keep_bf[:, :, b0:b0 + GB].rearrange("p j b -> p b j")
                    [:, :, :, None].to_broadcast([128, GB, 4, T]),
                op=mybir.AluOpType.mult,
            )
            prode = wpool.tile([128, GB, 4, T], BF16, tag="prode")
            nc.vector.tensor_tensor(
                out=prode[:], in0=em4[:, :, :, 0:T], in1=ohm16[:],
                op=mybir.AluOpType.mult,
            )
            prodr = wpool.tile([128, GB, 4, T], F32, tag="prodr")
            nc.vector.tensor_tensor(
                out=prodr[:],
                in0=r8[:].rearrange("p (b j) e -> p b j e", j=4)[:, :, :, 0:T],
                in1=oh16[:], op=mybir.AluOpType.mult,
            )
            for bi in range(GB):
                b = b0 + bi
                junk1 = wpool.tile([128, 4 * T], BF16, tag="junk1")
                nc.scalar.activation(
                    out=junk1[:], in_=prode[:, bi, :, :].rearrange("p a b -> p (a b)"),
                    func=mybir.ActivationFunctionType.Copy,
                    accum_out=estack[:, b:b + 1],
                )
                junk2 = wpool.tile([128, 4 * T], F32, tag="junk2")
                nc.scalar.activation(
                    out=junk2[:], in_=prodr[:, bi, :, :].rearrange("p a b -> p (a b)"),
                    func=mybir.ActivationFunctionType.Copy,
                    accum_out=tstack[:, b:b + 1],
                )
            # c[:, (b j)] = keep * (48*E + Tsum); E = table col 48
            t2 = wpool.tile([128, GB, 4], F32, tag="t2")
            nc.vector.tensor_scalar(
                out=t2[:], in0=em4[:, :, :, T], scalar1=float(T),
                scalar2=msc[:, 0:1], op0=mybir.AluOpType.mult,
                op1=mybir.AluOpType.add,
            )
            nc.vector.tensor_tensor(
                out=cstack[:, 4 * b0:4 * (b0 + GB)].rearrange(
                    "p (b j) -> p b j", j=4),
                in0=t2[:],
                in1=keep_f[:, :, b0:b0 + GB].rearrange("p j b -> p b j"),
                op=mybir.AluOpType.mult,
            )

        gstack = cpool.tile([128, BPC], F32)
        nc.vector.tensor_add(gstack[:], estack[:], tstack[:])

        # blocked scan: cT[4b+j, p] = c[b, 128j+p]; rows 4b scan t=0..127
        psc = pp.tile([128, 128], F32)
        nc.tensor.transpose(out=psc[:], in_=cstack[:], identity=identf[:])
        cT = cpool.tile([128, 128], F32)
        nc.vector.tensor_copy(cT[:], psc[:])
        scano = cpool.tile([128, 128], F32)
        nc.vector.tensor_tensor_scan(
            out=scano[:], data0=d48[:], data1=cT[:], initial=msc[:, 1:2],
            op0=mybir.AluOpType.mult, op1=mybir.AluOpType.add,
        )
        nc.sync.dma_start(logzraw[:, None], scano[:, 127:128])

        psg = pp.tile([1, BPC], F32)
        nc.tensor.matmul(psg[:], lhsT=ones[:], rhs=gstack[:], start=True,
                         stop=True)
        gold_sb = cpool.tile([1, BPC], F32)
        nc.vector.tensor_copy(gold_sb[:], psg[:])
        nc.sync.dma_start(gold[None, :], gold_sb[0:1, :])

    nc.compile()
    return nc


def _wrap16(vals):
    """dma_gather idx layout: idx i at [i%16, i//16], replicated to 128 rows."""
    n = vals.shape[0]
    w = np.zeros((16, n // 16), np.int16)
    w[np.arange(n) % 16, np.arange(n) // 16] = vals.astype(np.int16)
    return np.tile(w, (8, 1))


def _prep(inputs):
    """Host-side marshaling of the full inputs into per-core input maps."""
    f32 = np.float32
    seq = np.asarray(inputs["sequence"]).astype(np.int32)
    tgt = np.asarray(inputs["targets"]).astype(np.int32)
    emb = np.asarray(inputs["embedding"], f32)
    W_feat = np.asarray(inputs["W_feat"], f32)
    b_feat = np.asarray(inputs["b_feat"], f32)
    W_emit = np.asarray(inputs["W_emit"], f32)
    b_emit = np.asarray(inputs["b_emit"], f32)
    transitions = np.asarray(inputs["transitions"], f32)
    start_trans = np.asarray(inputs["start_trans"], f32)

    embp = np.zeros((VPAD, EMB), f32)
    embp[:VOCAB] = emb
    embT_bf = np.ascontiguousarray(embp.T).astype(BF16_NP)   # [256, VPAD]

    wfeat = np.ascontiguousarray(
        W_feat.reshape(2, 128, HID).transpose(1, 0, 2)).astype(BF16_NP)
    wemit_ext = np.zeros((HID, TBL), f32)
    wemit_ext[:, :T] = W_emit
    wemit_ext[:, T] = W_emit.sum(axis=1)
    wemit = np.ascontiguousarray(
        wemit_ext.reshape(4, 128, TBL).transpose(1, 0, 2)).astype(BF16_NP)
    bfeat = np.ascontiguousarray(b_feat.reshape(4, 128).T).astype(f32)
    bemit = np.zeros((TBL, 1), f32)
    bemit[:T, 0] = b_emit
    bemit[T, 0] = b_emit.sum()

    # transition banks: rows [0:48] = T, 48 = zeros, [49:97] = T + start, 97 = 0
    transp = np.zeros((2 * (T + 1), TBL), f32)
    transp[:T, :T] = transitions
    transp[T + 1:2 * T + 1, :T] = transitions + start_trans[None, :]
    misc = np.zeros((128, 2), f32)
    misc[:, 0] = transitions.sum()
    misc[:, 1] = start_trans.sum()

    in_maps_a = []
    in_maps_b = []
    for k in range(NCORES):
        in_maps_a.append({
            "embshT": np.ascontiguousarray(embT_bf[:, k * VSH:(k + 1) * VSH]),
            "wfeat": wfeat, "wemit": wemit, "bfeat": bfeat, "bemit": bemit,
        })
        sl = slice(k * BPC, (k + 1) * BPC)
        seq_c = seq[sl]                               # [32, 512]
        tgt_c = tgt[sl]
        ids = seq_c.reshape(-1).astype(np.int64)      # token i = b*512 + s
        lo = np.where(ids < HI0, ids, 0)
        hi = np.where(ids >= HI0, ids - HI0, 0)
        tn = np.empty_like(tgt_c)
        tn[:, :-1] = tgt_c[:, 1:]
        tn[:, 0] += T + 1                             # s=0: bank with +start
        tn[:, -1] = T                                 # s=511: zero row
        in_maps_b.append({
            "seqT": np.ascontiguousarray(seq_c.T),
            "tgtT": np.ascontiguousarray(tgt_c.T),
            "sqlo": _wrap16(lo), "sqhi": _wrap16(hi),
            "tgn": _wrap16(tn.reshape(-1)),
            "transp": transp, "misc": misc,
        })
    return in_maps_a, in_maps_b


def kernel(**inputs) -> np.ndarray:
    in_maps_a, in_maps_b = _prep(inputs)

    if "nca" not in _CACHE:
        _CACHE["nca"] = _build_phase_a()
    if "ncb" not in _CACHE:
        _CACHE["ncb"] = _build_phase_b()

    core_ids = list(range(NCORES))
    res_a = run_bass_kernel_spmd(_CACHE["nca"], in_maps_a, core_ids)
    tbl = np.concatenate(
        [np.asarray(res_a.results[k]["tablesh"]) for k in range(NCORES)],
        axis=0)                                   # [VPAD, 128] bf16

    for k in range(NCORES):
        in_maps_b[k]["table"] = tbl
    res_b = run_bass_kernel_spmd(_CACHE["ncb"], in_maps_b, core_ids)

    log_z = np.empty((B,), np.float32)
    goldv = np.empty((B,), np.float32)
    for k in range(NCORES):
        r = res_b.results[k]
        log_z[k * BPC:(k + 1) * BPC] = np.asarray(r["logzraw"])[0::4]
        goldv[k * BPC:(k + 1) * BPC] = np.asarray(r["gold"])
    return np.stack([log_z, goldv]).astype(np.float32)
